# revision 41
# baseline (speedup 1.0000x reference)
"""Grouped SwiGLU FFN (8 experts) — expert-parallel Bass kernel for 8 trn2 cores.

Per core (one expert): out = (silu(x@w1) * (x@w3T)) @ w2T.
  x: [T=1024, D=2048], w1: [D, H=4096], w3: [H, D], w2: [D, H].

Device-side formulation (fp16 matmul operands at full PE rate; zero
on-device transposes; layouts pre-packed on host):
  phase1: g^T[h, t]  = silu(w1^T-tile.T @ x^T) * (w3-tile.T @ x^T)  per h-tile,
          all 32 h-tiles kept resident in SBUF as fp16 (8 MB)
  phase2: out^T[d,t] = w2-tile.T @ g^T, one 32-matmul PSUM accumulation per
          (d-tile, t-half) — no SBUF accumulator, tail is one copy + DMA

Startup is gated by early DMA bandwidth (~175 GB/s while the rings spin
up), so the first four h-tiles run from fp8-e3m4 copies of x (x*2) and
w1/w3 (w*128) — same 1 cycle/row PE rate, half the startup bytes. The
SiLU input is descaled via the activation scale operand (1/256); the
gate output is stored as 256*g and the factor is folded into the
corresponding w2 rows on the host. Tiny "burn" matmuls on zeroed scratch
ramp the PE clock while the DMA rings initialize.
"""

import sys

sys.path.insert(0, "/opt/trn_rl_repo")

import numpy as np

import concourse.bass as bass
from concourse import bacc
import concourse.mybir as mybir
import concourse.tile as tile
from concourse.bass_utils import run_bass_kernel_spmd

E, T, D, H = 8, 1024, 2048, 4096
P = 128
NT = 512            # matmul moving free dim (ISA limit)
DT = D // P         # 16 contraction tiles over D
DP = DT // 2        # 8 dt-pairs (x is DMA'd in pairs: big partition lines)
HT = H // P         # 32 h-tiles
H8 = 2              # h-tiles computed from the fp8-e3m4 copies
TH = T // NT        # 2 t-halves
DTT = D // P        # 16 out^T row tiles
HD = DT // 2
NBURN = 42          # tiny clock-ramp matmuls issued before real work
NTB = 64            # burn matmul moving size
XS = 2.0            # e3m4 scale for x
WS = 128.0          # e3m4 scale for w1/w3
F32 = mybir.dt.float32
F16 = mybir.dt.float16
F8 = mybir.dt.float8e3

_CACHE: dict = {}


def _build_nc():
    nc = bacc.Bacc("TRN2", target_bir_lowering=False, debug=False)
    # merged inputs (runtime init costs ~0.3us per tensor):
    # warm8: [x8 pairs (DP) | w18 tiles (H8) | w38 tiles (H8)], e3m4
    # big16: [x pairs (DP) | w1/w3 tiles interleaved (2*HT)], fp16
    warm8 = nc.dram_tensor(
        "warm8", [DP + 2 * H8, P, 2 * T], F8, kind="ExternalInput"
    )
    # only h-tiles >= H8 load fp16 weights (the first H8 use the e3m4 copy)
    big16 = nc.dram_tensor(
        "big16", [DP + 2 * (HT - H8), P, 2 * T], F16, kind="ExternalInput"
    )
    w2p = nc.dram_tensor("w2p", [DTT, P, HT, P], F16, kind="ExternalInput")
    outT = nc.dram_tensor("outT", [D, T], F16, kind="ExternalOutput")

    with tile.TileContext(nc) as tc:
        with (
            tc.tile_pool(name="xpool", bufs=1) as xpool,
            tc.tile_pool(name="gpool", bufs=1) as gpool,
            tc.tile_pool(name="wpool", bufs=3) as wpool,
            tc.tile_pool(name="w8pool", bufs=4) as w8pool,
            tc.tile_pool(name="w2pool", bufs=3) as w2pool,
            tc.tile_pool(name="spool", bufs=1) as spool,
            tc.tile_pool(name="ospool", bufs=4) as ospool,
            tc.tile_pool(name="pspool", bufs=8, space="PSUM") as pspool,
        ):
            # --- PE clock pre-burn on zeroed scratch (no DMA dependency)
            burnw = spool.tile([P, P], F16, tag="burnw")
            burnx = spool.tile([P, NTB], F16, tag="burnx")
            nc.vector.memset(burnw, 0.0)
            nc.vector.memset(burnx, 0.0)
            psb = pspool.tile([P, NT], F32, tag="po", bufs=4, name="psburn")
            for i in range(NBURN):
                nc.tensor.matmul(
                    psb[:, 0:NTB],
                    lhsT=burnw,
                    rhs=burnx,
                    start=(i == 0),
                    stop=(i == NBURN - 1),
                )

            def load_w(ht):
                blk = DP + 2 * (ht - H8)
                w1sb = wpool.tile([P, DT, P], F16, tag="w1", name=f"w1sb_{ht}")
                nc.sync.dma_start(w1sb, big16[blk])
                w3sb = wpool.tile([P, DT, P], F16, tag="w3", name=f"w3sb_{ht}")
                nc.sync.dma_start(w3sb, big16[blk + 1])
                return w1sb, w3sb

            # --- startup DMAs in exact first-use order (early phase is
            # bandwidth-bound, so the warm set is the small e3m4 copy)
            w8sb = [
                (
                    w8pool.tile([P, DT, P], F8, tag="w18", name=f"w18sb_{k}"),
                    w8pool.tile([P, DT, P], F8, tag="w38", name=f"w38sb_{k}"),
                )
                for k in range(H8)
            ]
            xsb8 = xpool.tile([P, DT, T], F8, tag="x8")
            xsb = xpool.tile([P, DT, T], F16, tag="x")

            def w8dma(k, wi, half):
                sl = slice(half * HD, (half + 1) * HD)
                blk = DP + wi * H8 + k
                nc.sync.dma_start(
                    w8sb[k][wi][:, sl],
                    warm8[blk, :, half * T : (half + 1) * T],
                )

            def x8one(dt_i):
                k, j = divmod(dt_i, 2)
                nc.sync.dma_start(
                    xsb8[:, dt_i], warm8[k, :, j * T : (j + 1) * T]
                )

            w8dma(0, 0, 0)
            x8one(0)
            w8dma(0, 1, 0)
            w8dma(1, 0, 0)
            w8dma(1, 1, 0)
            for dt_i in (1, 2, 3):
                x8one(dt_i)
            for k in range(2):
                w8dma(k, 0, 1)
                w8dma(k, 1, 1)
            for k in range(2, DP):
                nc.sync.dma_start(xsb8[:, 2 * k : 2 * k + 2], warm8[k])
            for k in range(2, H8):
                nc.sync.dma_start(w8sb[k][0], warm8[DP + k])
                nc.sync.dma_start(w8sb[k][1], warm8[DP + H8 + k])
            # full-precision x for h-tiles >= H8
            for k in range(DP):
                nc.sync.dma_start(xsb[:, 2 * k : 2 * k + 2], big16[k])

            g = gpool.tile([P, HT, T], F16, tag="g")

            def epilogue(ps1, ps3, ht, th, scale):
                ts = slice(th * NT, (th + 1) * NT)
                sil = spool.tile([P, NT], F32, tag="sil")
                nc.scalar.activation(
                    sil, ps1, mybir.ActivationFunctionType.Silu, scale=scale
                )
                nc.vector.tensor_mul(out=g[:, ht, ts], in0=sil, in1=ps3)

            SC8 = 1.0 / (XS * WS)

            # --- phase 1 warm start: first two h-tiles interleaved across
            # 8 accumulation groups (all 8 psum banks), chunk-paced
            wgrp = []
            for ht in range(2):
                tag = "ps" if ht == 0 else "po"
                pairs = []
                for th in range(TH):
                    ps1 = pspool.tile([P, NT], F32, tag=tag, bufs=4, name="ps1")
                    ps3 = pspool.tile([P, NT], F32, tag=tag, bufs=4, name="ps3")
                    pairs.append((ps1, ps3))
                for th in range(TH):
                    wgrp.append((pairs[th][0], w8sb[ht][0], th, ht))
                for th in range(TH):
                    wgrp.append((pairs[th][1], w8sb[ht][1], th, ht))
            for dt_i in range(DT):
                for ps, wsb, th, _ht in wgrp:
                    ts = slice(th * NT, (th + 1) * NT)
                    nc.tensor.matmul(
                        ps,
                        lhsT=wsb[:, dt_i],
                        rhs=xsb8[:, dt_i, ts],
                        start=(dt_i == 0),
                        stop=(dt_i == DT - 1),
                    )
            for i in (0, 1, 4, 5):
                ps1, _, th, ht = wgrp[i]
                ps3 = wgrp[i + 2][0]
                epilogue(ps1, ps3, ht, th, SC8)

            for ht in range(2, HT):
                if ht < H8:
                    w1sb, w3sb = w8sb[ht]
                    xs_t, scale = xsb8, SC8
                else:
                    w1sb, w3sb = load_w(ht)
                    xs_t, scale = xsb, 1.0
                for th in range(TH):
                    ps1 = pspool.tile([P, NT], F32, tag="ps", bufs=4, name="ps1")
                    ps3 = pspool.tile([P, NT], F32, tag="ps", bufs=4, name="ps3")
                    ts = slice(th * NT, (th + 1) * NT)
                    for dt_i in range(DT):
                        nc.tensor.matmul(
                            ps1,
                            lhsT=w1sb[:, dt_i],
                            rhs=xs_t[:, dt_i, ts],
                            start=(dt_i == 0),
                            stop=(dt_i == DT - 1),
                        )
                    for dt_i in range(DT):
                        nc.tensor.matmul(
                            ps3,
                            lhsT=w3sb[:, dt_i],
                            rhs=xs_t[:, dt_i, ts],
                            start=(dt_i == 0),
                            stop=(dt_i == DT - 1),
                        )
                    epilogue(ps1, ps3, ht, th, scale)

            # --- phase 2: per (d-tile, t-half), one 32-matmul accumulation
            # over the whole H in a single psum bank, then copy + store.
            # w2 rows for the e3m4 h-tiles are pre-divided by XS*WS on host.
            for dtt in range(DTT):
                w2sb = w2pool.tile([P, HT, P], F16, tag="w2")
                nc.sync.dma_start(w2sb, w2p[dtt])
                for th in range(TH):
                    ts = slice(th * NT, (th + 1) * NT)
                    po = pspool.tile([P, NT], F32, tag="po", bufs=4, name="po")
                    for ht in range(HT):
                        nc.tensor.matmul(
                            po,
                            lhsT=w2sb[:, ht],
                            rhs=g[:, ht, ts],
                            start=(ht == 0),
                            stop=(ht == HT - 1),
                        )
                    osb = ospool.tile([P, NT], F16, tag="osb")
                    nc.vector.tensor_copy(out=osb, in_=po)
                    nc.sync.dma_start(
                        outT[dtt * P : (dtt + 1) * P, ts], osb
                    )
    nc.compile()
    return nc


def _pack_inputs(x, w1, w2, w3):
    """Per-expert host-side packing into DMA-linear layouts."""
    import ml_dtypes

    e3 = np.dtype(ml_dtypes.float8_e3m4)
    HC = H8 * P
    in_maps = []
    for e in range(E):
        x32 = np.asarray(x[e], dtype=np.float32)
        w132 = np.asarray(w1[e], dtype=np.float32)
        w332 = np.asarray(w3[e], dtype=np.float32)
        xe = x32.astype(np.float16)
        w1e = w132.astype(np.float16)
        w3e = w332.astype(np.float16)
        # w2 rows for the e3m4 h-tiles carry the 1/(XS*WS) fold
        w2s = np.asarray(w2[e], dtype=np.float32).copy()
        w2s[:, :HC] /= XS * WS
        w2e = w2s.astype(np.float16)
        # xp[k, p, j*T+t] = x[t, (2k+j)*128+p]  (dt-pair major)
        def packx(a):
            return np.ascontiguousarray(
                a.reshape(T, DP, 2, P).transpose(1, 3, 2, 0)
            ).reshape(DP, P, 2 * T)

        xp = packx(xe)
        xp8 = packx((x32 * XS).astype(e3))
        # w1p[ht, p, dt, h] = w1[dt*128+p, ht*128+h]
        w1p = w1e.reshape(DT, P, HT, P).transpose(2, 1, 0, 3)
        w18p = (
            (w132[:, :HC] * WS)
            .astype(e3)
            .reshape(DT, P, H8, P)
            .transpose(2, 1, 0, 3)
        )
        # w3p[ht, p, dt, h] = w3[ht*128+h, dt*128+p]
        w3p = w3e.reshape(HT, P, DT, P).transpose(0, 3, 2, 1)
        w38p = (
            (w332[:HC, :] * WS)
            .astype(e3)
            .reshape(H8, P, DT, P)
            .transpose(0, 3, 2, 1)
        )
        # merged tensors: warm8 = [x8 | w18 | w38], big16 = [x | w1/w3]
        warm8 = np.concatenate(
            [
                xp8,
                w18p.reshape(H8, P, 2 * T),
                w38p.reshape(H8, P, 2 * T),
            ],
            axis=0,
        )
        inter = np.stack(
            [
                w1p.reshape(HT, P, 2 * T)[H8:],
                w3p.reshape(HT, P, 2 * T)[H8:],
            ],
            axis=1,
        ).reshape(2 * (HT - H8), P, 2 * T)
        big16 = np.concatenate([xp, inter], axis=0)
        # w2p[dtt, p, ht, d] = w2[dtt*128+d, ht*128+p]
        w2p = np.ascontiguousarray(
            w2e.reshape(DTT, P, HT, P).transpose(0, 3, 2, 1)
        )
        in_maps.append(
            {
                "warm8": np.ascontiguousarray(warm8),
                "big16": np.ascontiguousarray(big16),
                "w2p": w2p,
            }
        )
    return in_maps


def kernel(x, w1, w2, w3, _trace=False, _trace_kwargs=None):
    if "nc" not in _CACHE:
        _CACHE["nc"] = _build_nc()
    nc = _CACHE["nc"]
    in_maps = _pack_inputs(x, w1, w2, w3)
    kw = {}
    if _trace:
        kw = {"trace": True}
        if _trace_kwargs:
            kw.update(_trace_kwargs)
    res = run_bass_kernel_spmd(nc, in_maps, core_ids=list(range(E)), **kw)
    out = np.empty((E, T, D), dtype=np.float32)
    for e in range(E):
        out[e] = res.results[e]["outT"].T.astype(np.float32)
    if _trace:
        _CACHE["last_results"] = res
    return out
